# revision 64
# baseline (speedup 1.0000x reference)
"""Trainium2 Bass kernel for nn_ActorCriticGAT (2-layer GATv2 + actor/critic heads).

The reference network's output is (action_logits[2], state_value[1]), both computed
solely from emb[node_to_assign_idx].  GATv2 layers couple nodes only through
per-destination segment softmax / segment sum over in-edges, so the output depends
exactly on the 2-hop in-neighborhood of node_to_assign_idx:

  layer-2 edges  E2 = { e : dst[e] == idx }              (~17 edges)
  needed h nodes S1 = {idx} U src[E2]                    (~18 nodes)
  layer-1 edges  E1 = { e : dst[e] in S1 }               (~310 edges)

Host side (numpy) does only index work — boolean masks, gathers of x rows, and
one-hot scatter/gather matrices (the same work a DMA engine's descriptors would
do).  All model arithmetic — the linear layers, leaky-relu, per-segment softmax,
alpha-weighted aggregation, elu, and both MLP heads — runs on the NeuronCores as
TensorE matmuls + Vector/Scalar engine elementwise ops.  Segment softmax/scatter
are expressed as one-hot matmuls (A1^T @ .), so no indirect addressing is needed
on-device.

All device constants are packed host-side into a handful of contiguous blocks,
grouped by row count and by when the compute needs them, and loaded with one
dma_start each (large transfers parallelize across all 16 SDMA engines; many
small DMAs each pay the ~2us fixed completion cost).  The pack feeding the
first matmul chain is shipped first so TensorE starts as early as possible.

Matmul inputs are bf16 (PSUM accumulation and all Vector/Scalar-engine math
stay fp32) — this halves both the weight-DMA bytes and the TensorE cost and
costs ~1.5e-3 relative error against the fp32 reference, far inside the 2e-2
accuracy gate.

Numerical notes (exact or negligible vs the reference):
 - exp() without max-subtraction: logits here are O(1), and alpha is the same
   rational function of the logits either way (the 1e-16 epsilon shifts by
   exp(-amax), which is negligible at these magnitudes).
 - division uses (denom + 1e-16), matching the reference formula exactly
   (dropped for the layer-2 softmax when real in-edges exist: denom >= exp(min
   logit) >> 1e-16 makes it a strict no-op there).
 - leaky_relu(x) = 0.2x + relu(0.8x) and elu(x) = max(x,0) + min(exp(x),1) - 1
   are exact rewrites chosen for engine balance.
 - linear-layer biases that feed the alpha-weighted aggregation are applied
   after aggregation (bias * sum(alpha) == bias up to the 1e-16 epsilon); for
   this model every such bias is exactly zero anyway.

The work is replicated SPMD across all 8 NeuronCores (the pruned subgraph is far
below one core's granularity, so partitioning it would only add collective
latency); core 0's output is returned.
"""
import numpy as np

N_NODES = 50000
D = 128          # input feature dim
C = 128          # channels per head
H = 4            # heads, layer 1
HC = H * C       # 512
MLP = 64


BF16_L1 = True   # layer-1 edge matmuls in bf16 (PSUM accumulation stays fp32)


class _Packer:
    """Pack same-row-count tensors side by side into one [rows, W] block so a
    single dma_start ships them (large DMAs parallelize across all 16 SDMA
    engines; many small DMAs each pay the ~2us fixed completion cost)."""

    def __init__(self, rows, dtype=np.float32):
        self.rows = rows
        self.dtype = dtype
        self.entries = {}
        self.w = 0

    def add(self, name, arr):
        r, c = arr.shape
        assert r <= self.rows
        self.entries[name] = (self.w, r, c, arr)
        self.w += c

    def finalize(self):
        out = np.zeros((self.rows, self.w), np.float32)
        lay = {}
        for name, (off, r, c, arr) in self.entries.items():
            out[:r, off:off + c] = arr
            lay[name] = (off, r, c)
        return out.astype(self.dtype), lay


def _build(nc, dims, lays):
    """Build the Bass/Tile graph.  dims = (E1p, ec, n1, E2, degenerate)."""
    import concourse.bass as bass
    import concourse.tile as tile
    from concourse import mybir
    from concourse.masks import make_identity

    F32 = mybir.dt.float32
    F16 = mybir.dt.bfloat16
    L1DT = F16 if BF16_L1 else F32
    E1p, ec, n1, E2, degenerate = dims
    layA, layB, layR, layG, layM, layW, att_w = lays

    def mega_param(name, rows, lay, dt=F32):
        w = max(o + c for o, _, c in lay.values())
        return nc.declare_dram_parameter(name, [rows, w], dt, isOutput=False)

    megaA_d = mega_param("megaA", 128, layA, L1DT)
    megaB_d = mega_param("megaB", 128, layB, L1DT)
    megaR_d = mega_param("megaR", 2, layR, L1DT)
    megaG_d = mega_param("megaG", n1, layG, L1DT)
    megaM_d = mega_param("megaM", 128, layM)
    megaW_d = mega_param("megaW", 128, layW, L1DT)
    att_d = nc.declare_dram_parameter("attrow", [1, att_w], F32, isOutput=False)
    out_d = nc.declare_dram_parameter("out", [3, 1], F32, isOutput=True)

    AL = mybir.AluOpType
    ACT = mybir.ActivationFunctionType

    def bview(ap, inner):
        """Append a stride-0 inner free dim (broadcast) to an AP."""
        return bass.AP(tensor=ap.tensor, offset=ap.offset, ap=[*ap.ap, [0, inner]])

    with tile.TileContext(nc) as tc:
        with (
            tc.tile_pool(name="const", bufs=1) as cs,
            tc.tile_pool(name="work", bufs=2) as wk,
            tc.tile_pool(name="chunk", bufs=max(ec, 1)) as ck,
            tc.tile_pool(name="psA", bufs=3, space="PSUM") as psA,
            tc.tile_pool(name="psB", bufs=1, space="PSUM") as psB,
            tc.tile_pool(name="psC", bufs=3, space="PSUM") as psC,
        ):
            # warm the ScalarE activation table (Exp) while DMAs run
            warm = cs.tile([1, 1], F32, tag="warm")
            nc.vector.memset(warm[:], 0.0)
            nc.scalar.activation(warm[:], warm[:], ACT.Exp)

            tA = cs.tile([128, megaA_d.shape[1]], L1DT, tag="tA")
            tB = cs.tile([128, megaB_d.shape[1]], L1DT, tag="tB")
            tR = cs.tile([2, megaR_d.shape[1]], L1DT, tag="tR")
            tG = cs.tile([n1, megaG_d.shape[1]], L1DT, tag="tG")
            tM = cs.tile([128, megaM_d.shape[1]], F32, tag="tM")
            tW = cs.tile([128, megaW_d.shape[1]], L1DT, tag="tW")
            tAtt = cs.tile([128, att_w], F32, tag="tAtt")
            nc.sync.dma_start(out=tR[:], in_=megaR_d[:])
            nc.sync.dma_start(out=tA[:], in_=megaA_d[:])
            nc.sync.dma_start(out=tB[:], in_=megaB_d[:])
            nc.sync.dma_start(out=tG[:], in_=megaG_d[:])
            nc.sync.dma_start(out=tAtt[:], in_=att_d[:].to_broadcast([128, att_w]))
            nc.sync.dma_start(out=tM[:], in_=megaM_d[:])
            nc.sync.dma_start(out=tW[:], in_=megaW_d[:])

            def sl(tile_, lay, name):
                off, r, c = lay[name]
                return tile_[:r, off:off + c]

            xsT0 = sl(tA, layA, "xsT0")
            Wl1 = sl(tA, layA, "Wl1")
            xsT12 = sl(tB, layB, "xsT12") if ec > 1 else None
            xdT = sl(tB, layB, "xdT")
            Wr1 = sl(tB, layB, "Wr1")
            A1T = sl(tW, layW, "A1T")
            bias1T4 = sl(tM, layM, "bias1T4")
            wext = sl(tR, layR, "wext")
            we1b = sl(tR, layR, "we1b")
            w2ext = sl(tR, layR, "w2ext")
            we2b = sl(tR, layR, "we2b")
            G2T = sl(tG, layG, "G2T")
            att1b = tAtt[:, 0:HC]
            att2b = tAtt[:, HC:HC + C]
            Wlr2 = sl(tW, layW, "Wlr2")
            bias2c = sl(tM, layM, "bias2col")
            Wpv1 = sl(tM, layM, "Wpv1")
            bpv1c = sl(tM, layM, "bpv1col")
            Wout = sl(tM, layM, "Wout")
            boutc = sl(tM, layM, "boutcol")
            mask2 = sl(tM, layM, "mask2") if degenerate else None

            ident = cs.tile([128, 128], F32, tag="ident")
            make_identity(nc, ident[:])
            ones_row = cs.tile([1, 128], F32, tag="ones_row")
            nc.vector.memset(ones_row[:], 1.0)
            ones_col = cs.tile([128, 1], F32, tag="ones_col")
            nc.vector.memset(ones_col[:], 1.0)

            # ---- layer 1, per 128-edge chunk ----
            # engine balance: TensorE 4 bf16 matmuls; ScalarE psum-copy + exp;
            # GpSimd att-product; DVE leaky-relu, reduce, p-weighting.
            p_list, wgt_list = [], []
            for k in range(ec):
                ks = slice(k * 128, (k + 1) * 128)
                xsTk = xsT0 if k == 0 else xsT12[:, (k - 1) * 128:k * 128]
                # pb = xl + xr + e + (bl1+br1)  (emitted first: the DVE chain
                # hangs off it, while pa is only needed late, by the weighting)
                pb = psA.tile([128, HC], F32, tag="ps")
                nc.tensor.matmul(pb[:], xsTk, Wl1[:], start=True, stop=False)
                nc.tensor.matmul(pb[:], xdT[:, ks], Wr1[:], start=False, stop=False)
                nc.tensor.matmul(pb[:], wext[:, ks], we1b[:], start=False, stop=True)
                # pa = xl = x_src @ Wl1 (raw; kept in PSUM for the p-weighting)
                pa = psA.tile([128, HC], F32, tag="ps")
                nc.tensor.matmul(pa[:], xsTk, Wl1[:], start=True, stop=True)
                # m = leaky_relu(pb, 0.2) = 0.2*pb + relu(0.8*pb)
                # (relu term on ScalarE with fused scale; one DVE op from PSUM)
                r8 = wk.tile([128, HC], F32, tag="r8")
                nc.scalar.activation(r8[:], pb[:], ACT.Relu, scale=0.8)
                m = wk.tile([128, HC], F32, tag="m")
                nc.vector.scalar_tensor_tensor(
                    out=m[:], in0=pb[:], scalar=0.2, in1=r8[:],
                    op0=AL.mult, op1=AL.add)
                # logits[e,h] = sum_c m[e, h*128+c] * att1[h,c];  p = exp(logits)
                prod = wk.tile([128, HC], F32, tag="prod")
                nc.vector.tensor_mul(prod[:], m[:], att1b[:])
                logit = wk.tile([128, H], F32, tag="logit")
                nc.vector.tensor_reduce(
                    logit[:], prod[:].rearrange("e (h c) -> e h c", h=H),
                    mybir.AxisListType.X, AL.add)
                p = wk.tile([128, H], F32, tag="pf32")
                nc.scalar.activation(p[:], logit[:], ACT.Exp)
                p16 = ck.tile([128, H], F16, tag="p16")
                nc.gpsimd.tensor_copy(p16[:], p[:])
                # weighted source features: wgt = xl * p[e, h] (broadcast over c)
                wgt = ck.tile([128, HC], F16, tag="wgt")
                nc.vector.tensor_mul(
                    wgt[:].rearrange("e (h c) -> e h c", h=H),
                    pa[:].rearrange("e (h c) -> e h c", h=H),
                    bview(p[:], C))
                p_list.append(p16); wgt_list.append(wgt)

            # ---- segment denominators + aggregation via one-hot matmuls ----
            pden = psB.tile([128, H], F32, tag="pden")
            for k in range(ec):
                nc.tensor.matmul(pden[:n1, :], A1T[:, k * n1:(k + 1) * n1],
                                 p_list[k][:], start=(k == 0), stop=(k == ec - 1))
            phag = psB.tile([128, HC], F32, tag="phag")
            for k in range(ec):
                nc.tensor.matmul(phag[:n1, :], A1T[:, k * n1:(k + 1) * n1],
                                 wgt_list[k][:], start=(k == 0), stop=(k == ec - 1))
            rec = wk.tile([128, H], F32, tag="rec")
            nc.vector.tensor_scalar_add(rec[:n1, :], pden[:n1, :], 1e-16)
            nc.vector.reciprocal(rec[:n1, :], rec[:n1, :])

            # ---- per head-block: divide by denom, transpose -> hT, fused bias
            # add, elu, then immediately the layer-2 matmul for that block
            # (pipelined: block k's elu overlaps block k-1's matmul) ----
            hsb = wk.tile([128, HC], F32, tag="hsb")
            hT = wk.tile([128, H * n1], F16, tag="hT")
            pxlr2 = psC.tile([128, 2 * C], F32, tag="psc")
            for k in range(H):
                kb = slice(k * n1, (k + 1) * n1)
                hs = slice(k * 128, (k + 1) * 128)
                nc.vector.tensor_scalar_mul(
                    out=hsb[:n1, hs], in0=phag[:n1, hs],
                    scalar1=rec[:n1, k:k + 1])
                pt = psC.tile([128, 128], F32, tag="psc")
                nc.tensor.transpose(pt[:, :n1], hsb[:n1, hs],
                                    ident[:n1, :n1])
                # copy + per-partition bias in one op
                nc.vector.tensor_scalar_add(hT[:, kb], pt[:, :n1],
                                            bias1T4[:, k:k + 1])
                # elu(x) = max(x,0) + (min(exp(x),1) - 1)
                # (exp runs concurrently with the max on the other engine)
                t1 = wk.tile([128, n1], F32, tag="t1")
                nc.scalar.activation(t1[:], hT[:, kb], ACT.Exp)
                t2 = wk.tile([128, n1], F32, tag="t2")
                nc.vector.tensor_scalar(
                    out=t2[:], in0=hT[:, kb], scalar1=0.0, scalar2=-1.0,
                    op0=AL.max, op1=AL.add)
                nc.vector.scalar_tensor_tensor(
                    out=hT[:, kb], in0=t1[:], scalar=1.0, in1=t2[:],
                    op0=AL.min, op1=AL.add)
                # layer-2 node linears: one chain computes [xl2 | xr2]
                # (biases folded: bl2 -> bias2col, br2 -> we2b)
                nc.tensor.matmul(pxlr2[:n1, :], hT[:, kb],
                                 Wlr2[:, k * 256:(k + 1) * 256],
                                 start=(k == 0), stop=(k == H - 1))
            xl2 = wk.tile([128, C], F16, tag="xl2")
            nc.vector.tensor_copy(xl2[:n1, :], pxlr2[:n1, 0:C])
            xr2r = wk.tile([1, C], F32, tag="xr2r")
            nc.scalar.copy(xr2r[:], pxlr2[0:1, C:2 * C])

            # ---- layer 2 per-edge attention (single segment: dst == idx) ----
            pxle = psC.tile([128, C], F32, tag="psc")
            nc.tensor.matmul(pxle[:E2, :], G2T, xl2[:n1, :],
                             start=True, stop=True)
            xle = wk.tile([128, C], F32, tag="xle")
            nc.vector.tensor_copy(xle[:E2, :], pxle[:E2, :])
            # pm2 = xle + xr2[idx] + e2 + (bl2+br2), all accumulated on PSUM
            pm2 = psC.tile([128, C], F32, tag="psc")
            nc.tensor.matmul(pm2[:E2, :], G2T, xl2[:n1, :],
                             start=True, stop=False)
            nc.tensor.matmul(pm2[:E2, :], ones_row[:, :E2], xr2r[:],
                             start=False, stop=False)
            nc.tensor.matmul(pm2[:E2, :], w2ext, we2b,
                             start=False, stop=True)
            # m2 = leaky_relu(pm2, 0.2) = 0.2*pm2 + relu(0.8*pm2)
            r82 = wk.tile([128, C], F32, tag="r82")
            nc.scalar.activation(r82[:E2, :], pm2[:E2, :], ACT.Relu, scale=0.8)
            m2 = wk.tile([128, C], F32, tag="m2")
            nc.vector.scalar_tensor_tensor(
                out=m2[:E2, :], in0=pm2[:E2, :], scalar=0.2, in1=r82[:E2, :],
                op0=AL.mult, op1=AL.add)
            prod2 = wk.tile([128, C], F32, tag="prod2")
            nc.vector.tensor_mul(prod2[:E2, :], m2[:E2, :], att2b[:E2, :])
            logit2 = wk.tile([128, 1], F32, tag="logit2")
            nc.vector.tensor_reduce(logit2[:E2, :], prod2[:E2, :],
                                    mybir.AxisListType.X, AL.add)
            p2 = wk.tile([128, 1], F32, tag="p2")
            nc.scalar.activation(p2[:E2, :], logit2[:E2, :], ACT.Exp)
            if degenerate:
                nc.vector.tensor_mul(p2[:E2, :], p2[:E2, :], mask2)
            # zT_raw = xle^T @ p2 ; denom broadcast back via K=1 matmul
            pd2 = psC.tile([1, 1], F32, tag="psc")
            nc.tensor.matmul(pd2[:, :], p2[:E2, :], ones_col[:E2, :],
                             start=True, stop=True)
            pzr = psC.tile([128, 1], F32, tag="psc")
            nc.tensor.matmul(pzr[:, :], xle[:E2, :], p2[:E2, :],
                             start=True, stop=True)
            d2 = wk.tile([1, 1], F32, tag="d2")
            if degenerate:
                # masked-out edges can make denom exactly 0; keep the epsilon
                nc.vector.tensor_scalar_add(d2[:], pd2[:], 1e-16)
                nc.vector.reciprocal(d2[:], d2[:])
            else:
                # denom >= exp(min logit) >> 1e-16: epsilon is a strict no-op
                nc.vector.reciprocal(d2[:], pd2[:])
            prb = psC.tile([128, 1], F32, tag="psc")
            nc.tensor.matmul(prb[:, :], ones_row[:], d2[:], start=True, stop=True)
            rb = wk.tile([128, 1], F32, tag="rb")
            nc.vector.tensor_copy(rb[:], prb[:])
            zT = wk.tile([128, 1], F32, tag="zT")
            nc.vector.scalar_tensor_tensor(
                out=zT[:], in0=pzr[:], scalar=rb[:], in1=bias2c,
                op0=AL.mult, op1=AL.add)

            # ---- actor/critic heads, fully in transposed layout ----
            ph = psC.tile([128, 1], F32, tag="psc")
            nc.tensor.matmul(ph[:, :], Wpv1, zT[:], start=True, stop=True)
            hidT = wk.tile([128, 1], F32, tag="hidT")
            nc.vector.tensor_scalar(
                out=hidT[:], in0=ph[:], scalar1=bpv1c, scalar2=0.0,
                op0=AL.add, op1=AL.max)
            po = psC.tile([3, 1], F32, tag="psc")
            nc.tensor.matmul(po[:, :], Wout, hidT[:], start=True, stop=True)
            osb = wk.tile([3, 1], F32, tag="osb")
            nc.vector.tensor_scalar(
                out=osb[:], in0=po[:], scalar1=boutc, scalar2=None,
                op0=AL.add)
            nc.sync.dma_start(out=out_d[:], in_=osb[:])
    return nc


def _prepare(inputs):
    """Host-side exact pruning + operand layout.  Returns (dev_inputs, dims, lays)."""
    x = np.asarray(inputs["x"], np.float32)
    ei = np.asarray(inputs["edge_index"]).astype(np.int64)
    ew = np.asarray(inputs["edge_weight"], np.float32).reshape(-1)
    idx = int(np.asarray(inputs["node_to_assign_idx"]))
    src, dst = ei[0], ei[1]
    n_nodes = x.shape[0]

    e2_mask = dst == idx
    src2 = src[e2_mask]
    w2 = ew[e2_mask]
    E2 = int(src2.shape[0])
    degenerate = E2 == 0
    if degenerate:  # keep shapes >=1; contribution masked to zero on device
        src2 = np.array([idx]); w2 = np.zeros(1, np.float32)
        E2 = 1

    rest = np.unique(src2)
    rest = rest[rest != idx]
    S1 = np.concatenate([np.array([idx], np.int64), rest.astype(np.int64)])
    n1 = int(S1.shape[0])

    in_S1 = np.zeros(n_nodes, bool)
    in_S1[S1] = True
    e1_mask = in_S1[dst]
    src1, dst1, w1 = src[e1_mask], dst[e1_mask], ew[e1_mask]
    E1 = int(src1.shape[0])
    E1p = max(128, ((E1 + 127) // 128) * 128)
    ec = E1p // 128

    pos1 = np.full(n_nodes, -1, np.int64)
    pos1[S1] = np.arange(n1)

    xsT = np.zeros((128, E1p), np.float32)
    xsT[:, :E1] = x[src1].T
    xdT = np.zeros((128, E1p), np.float32)
    xdT[:, :E1] = x[dst1].T
    wext = np.zeros((2, E1p), np.float32)
    wext[0, :] = 1.0
    wext[1, :E1] = w1

    A1T = np.zeros((128, ec * n1), np.float32)
    e_ids = np.arange(E1)
    A1T[e_ids % 128, (e_ids // 128) * n1 + pos1[dst1]] = 1.0

    G2T = np.zeros((n1, E2), np.float32)
    G2T[pos1[src2], np.arange(E2)] = 1.0
    w2ext = np.stack([np.ones(E2, np.float32), w2.astype(np.float32)])

    g = lambda k: np.asarray(inputs[k], np.float32)
    # [128, H, 2C]: per k-block the columns are [Wl2 block | Wr2 block]
    Wlr2 = np.concatenate(
        [g("Wl2").reshape(H, 128, C).transpose(1, 0, 2),
         g("Wr2").reshape(H, 128, C).transpose(1, 0, 2)],
        axis=2).reshape(128, H * 2 * C)

    import ml_dtypes
    l1dt = ml_dtypes.bfloat16 if BF16_L1 else np.float32
    pa = _Packer(128, l1dt)
    pa.add("xsT0", xsT[:, :128])
    pa.add("Wl1", g("Wl1"))
    pb = _Packer(128, l1dt)
    if ec > 1:
        pb.add("xsT12", xsT[:, 128:])
    pb.add("xdT", xdT)
    pb.add("Wr1", g("Wr1"))
    pr = _Packer(2, l1dt)
    pr.add("wext", wext)
    pr.add("we1b", np.stack([g("bl1") + g("br1"), g("We1").reshape(-1)]))
    pr.add("w2ext", w2ext)
    pr.add("we2b", np.stack([g("bl2") + g("br2"), g("We2").reshape(-1)]))
    pg = _Packer(n1, l1dt)
    pg.add("G2T", G2T)
    pm = _Packer(128)
    pm.add("bias1T4", np.ascontiguousarray((g("bias1") + g("bl1")).reshape(H, 128).T))
    pm.add("Wpv1", np.concatenate([g("Wp1"), g("Wv1")], axis=1))
    pm.add("Wout", np.concatenate(
        [np.concatenate([g("Wp2"), np.zeros((MLP, 1), np.float32)], axis=1),
         np.concatenate([np.zeros((MLP, 2), np.float32), g("Wv2")], axis=1)]))
    pm.add("bias2col", (g("bias2") + g("bl2")).reshape(128, 1))
    pm.add("bpv1col", np.concatenate([g("bp1"), g("bv1")]).reshape(128, 1))
    pm.add("boutcol", np.concatenate([g("bp2"), g("bv2")]).reshape(3, 1))
    if degenerate:
        pm.add("mask2", np.zeros((E2, 1), np.float32))
    pw = _Packer(128, l1dt)
    pw.add("A1T", A1T)
    pw.add("Wlr2", Wlr2)

    attrow = np.concatenate(
        [g("att1").reshape(-1), g("att2").reshape(-1)]).reshape(1, HC + C)

    megaA, layA = pa.finalize()
    megaB, layB = pb.finalize()
    megaR, layR = pr.finalize()
    megaG, layG = pg.finalize()
    megaM, layM = pm.finalize()
    megaW, layW = pw.finalize()
    dev = {"megaA": megaA, "megaB": megaB, "megaR": megaR, "megaG": megaG,
           "megaM": megaM, "megaW": megaW, "attrow": attrow}
    return (dev, (E1p, ec, n1, E2, degenerate),
            (layA, layB, layR, layG, layM, layW, HC + C))


def _numpy_fallback(inputs):
    """Exact reference math in numpy (used only if the subgraph exceeds the
    single-tile device layout, which cannot happen for the problem's data)."""
    x = np.asarray(inputs["x"], np.float32)
    ei = np.asarray(inputs["edge_index"]).astype(np.int64)
    ew = np.asarray(inputs["edge_weight"], np.float32)
    idx = int(np.asarray(inputs["node_to_assign_idx"]))
    src, dst = ei[0], ei[1]
    n = x.shape[0]
    g = lambda k: np.asarray(inputs[k], np.float32)

    def layer(xf, Wl, bl, Wr, br, We, att, bias, heads, ch, concat):
        xl = (xf @ Wl + bl).reshape(-1, heads, ch)
        xr = (xf @ Wr + br).reshape(-1, heads, ch)
        e = (ew @ We).reshape(-1, heads, ch)
        m = xl[src] + xr[dst] + e
        m = np.where(m > 0, m, 0.2 * m)
        logits = np.einsum("ehc,hc->eh", m, att.reshape(heads, ch))
        amax = np.full((n, heads), -np.inf, np.float32)
        np.maximum.at(amax, dst, logits)
        amax = np.where(np.isfinite(amax), amax, 0.0)
        p = np.exp(logits - amax[dst])
        den = np.zeros((n, heads), np.float32)
        np.add.at(den, dst, p)
        alpha = p / (den[dst] + 1e-16)
        out = np.zeros((n, heads, ch), np.float32)
        np.add.at(out, dst, xl[src] * alpha[..., None])
        out = out.reshape(n, heads * ch) if concat else out.mean(1)
        return out + bias

    h = layer(x, g("Wl1"), g("bl1"), g("Wr1"), g("br1"), g("We1"), g("att1"),
              g("bias1"), H, C, True)
    h = np.where(h > 0, h, np.exp(np.minimum(h, 0)) - 1)
    emb = layer(h, g("Wl2"), g("bl2"), g("Wr2"), g("br2"), g("We2"), g("att2"),
                g("bias2"), 1, C, False)
    z = emb[idx]
    a = np.maximum(z @ g("Wp1") + g("bp1"), 0) @ g("Wp2") + g("bp2")
    v = np.maximum(z @ g("Wv1") + g("bv1"), 0) @ g("Wv2") + g("bv2")
    return a.astype(np.float32), v.astype(np.float32)


def kernel(**inputs):
    dev, dims, lays = _prepare(inputs)
    E1p, ec, n1, E2, degenerate = dims
    if n1 > 128 or E2 > 128:
        return _numpy_fallback(inputs)

    import concourse.bacc as bacc
    from concourse.bass_utils import run_bass_kernel_spmd

    nc = bacc.Bacc("TRN2", target_bir_lowering=False, debug=False)
    _build(nc, dims, lays)
    nc.compile()
    res = run_bass_kernel_spmd(nc, [dict(dev) for _ in range(8)], list(range(8)))
    out = np.asarray(res.results[0]["out"], np.float32).reshape(3)
    return out[:2].copy(), out[2:3].copy()


# revision 69
# speedup vs baseline: 1.0104x; 1.0104x over previous
"""Trainium2 Bass kernel for nn_ActorCriticGAT (2-layer GATv2 + actor/critic heads).

The reference network's output is (action_logits[2], state_value[1]), both computed
solely from emb[node_to_assign_idx].  GATv2 layers couple nodes only through
per-destination segment softmax / segment sum over in-edges, so the output depends
exactly on the 2-hop in-neighborhood of node_to_assign_idx:

  layer-2 edges  E2 = { e : dst[e] == idx }              (~17 edges)
  needed h nodes S1 = {idx} U src[E2]                    (~18 nodes)
  layer-1 edges  E1 = { e : dst[e] in S1 }               (~310 edges)

Host side (numpy) does only index work — boolean masks, gathers of x rows, and
one-hot scatter/gather matrices (the same work a DMA engine's descriptors would
do).  All model arithmetic — the linear layers, leaky-relu, per-segment softmax,
alpha-weighted aggregation, elu, and both MLP heads — runs on the NeuronCores as
TensorE matmuls + Vector/Scalar engine elementwise ops.  Segment softmax/scatter
are expressed as one-hot matmuls (A1^T @ .), so no indirect addressing is needed
on-device.

All device constants are packed host-side into a handful of contiguous blocks,
grouped by row count and by when the compute needs them, and loaded with one
dma_start each (large transfers parallelize across all 16 SDMA engines; many
small DMAs each pay the ~2us fixed completion cost).  The pack feeding the
first matmul chain is shipped first so TensorE starts as early as possible.

Matmul inputs are bf16 (PSUM accumulation and all Vector/Scalar-engine math
stay fp32) — this halves both the weight-DMA bytes and the TensorE cost and
costs ~1.5e-3 relative error against the fp32 reference, far inside the 2e-2
accuracy gate.

Numerical notes (exact or negligible vs the reference):
 - exp() without max-subtraction: logits here are O(1), and alpha is the same
   rational function of the logits either way (the 1e-16 epsilon shifts by
   exp(-amax), which is negligible at these magnitudes).
 - division uses (denom + 1e-16), matching the reference formula exactly
   (dropped for the layer-2 softmax when real in-edges exist: denom >= exp(min
   logit) >> 1e-16 makes it a strict no-op there).
 - leaky_relu(x) = 0.2x + relu(0.8x) and elu(x) = max(x,0) + min(exp(x),1) - 1
   are exact rewrites chosen for engine balance.
 - linear-layer biases that feed the alpha-weighted aggregation are applied
   after aggregation (bias * sum(alpha) == bias up to the 1e-16 epsilon); for
   this model every such bias is exactly zero anyway.

The work is replicated SPMD across all 8 NeuronCores (the pruned subgraph is far
below one core's granularity, so partitioning it would only add collective
latency); core 0's output is returned.
"""
import numpy as np

N_NODES = 50000
D = 128          # input feature dim
C = 128          # channels per head
H = 4            # heads, layer 1
HC = H * C       # 512
MLP = 64


BF16_L1 = True   # layer-1 edge matmuls in bf16 (PSUM accumulation stays fp32)


class _Packer:
    """Pack same-row-count tensors side by side into one [rows, W] block so a
    single dma_start ships them (large DMAs parallelize across all 16 SDMA
    engines; many small DMAs each pay the ~2us fixed completion cost)."""

    def __init__(self, rows, dtype=np.float32):
        self.rows = rows
        self.dtype = dtype
        self.entries = {}
        self.w = 0

    def add(self, name, arr):
        r, c = arr.shape
        assert r <= self.rows
        self.entries[name] = (self.w, r, c, arr)
        self.w += c

    def finalize(self):
        out = np.zeros((self.rows, self.w), np.float32)
        lay = {}
        for name, (off, r, c, arr) in self.entries.items():
            out[:r, off:off + c] = arr
            lay[name] = (off, r, c)
        return out.astype(self.dtype), lay


def _build(nc, dims, lays):
    """Build the Bass/Tile graph.  dims = (E1p, ec, n1, E2, degenerate)."""
    import concourse.bass as bass
    import concourse.tile as tile
    from concourse import mybir
    from concourse.masks import make_identity

    F32 = mybir.dt.float32
    F16 = mybir.dt.bfloat16
    L1DT = F16 if BF16_L1 else F32
    E1p, ec, n1, E2, degenerate = dims
    layA, layB, layR, layG, layM, layW, att_w = lays

    def mega_param(name, rows, lay, dt=F32):
        w = max(o + c for o, _, c in lay.values())
        return nc.declare_dram_parameter(name, [rows, w], dt, isOutput=False)

    megaA_d = mega_param("megaA", 128, layA, L1DT)
    megaB_d = mega_param("megaB", 128, layB, L1DT)
    megaR_d = mega_param("megaR", 2, layR, L1DT)
    megaG_d = mega_param("megaG", n1, layG, L1DT)
    megaM_d = mega_param("megaM", 128, layM)
    megaW_d = mega_param("megaW", 128, layW, L1DT)
    att_d = nc.declare_dram_parameter("attrow", [1, att_w], F32, isOutput=False)
    out_d = nc.declare_dram_parameter("out", [3, 1], F32, isOutput=True)

    AL = mybir.AluOpType
    ACT = mybir.ActivationFunctionType

    def bview(ap, inner):
        """Append a stride-0 inner free dim (broadcast) to an AP."""
        return bass.AP(tensor=ap.tensor, offset=ap.offset, ap=[*ap.ap, [0, inner]])

    with tile.TileContext(nc) as tc:
        with (
            tc.tile_pool(name="const", bufs=1) as cs,
            tc.tile_pool(name="work", bufs=2) as wk,
            tc.tile_pool(name="chunk", bufs=max(ec, 1)) as ck,
            tc.tile_pool(name="psA", bufs=3, space="PSUM") as psA,
            tc.tile_pool(name="psB", bufs=1, space="PSUM") as psB,
            tc.tile_pool(name="psC", bufs=3, space="PSUM") as psC,
        ):
            # warm the ScalarE activation table (Exp) while DMAs run
            warm = cs.tile([1, 1], F32, tag="warm")
            nc.vector.memset(warm[:], 0.0)
            nc.scalar.activation(warm[:], warm[:], ACT.Exp)

            tA = cs.tile([128, megaA_d.shape[1]], L1DT, tag="tA")
            tB = cs.tile([128, megaB_d.shape[1]], L1DT, tag="tB")
            tR = cs.tile([2, megaR_d.shape[1]], L1DT, tag="tR")
            tG = cs.tile([n1, megaG_d.shape[1]], L1DT, tag="tG")
            tM = cs.tile([128, megaM_d.shape[1]], F32, tag="tM")
            tW = cs.tile([128, megaW_d.shape[1]], L1DT, tag="tW")
            tAtt = cs.tile([128, att_w], F32, tag="tAtt")
            nc.sync.dma_start(out=tR[:], in_=megaR_d[:])
            nc.sync.dma_start(out=tA[:], in_=megaA_d[:])
            nc.sync.dma_start(out=tB[:], in_=megaB_d[:])
            nc.sync.dma_start(out=tG[:], in_=megaG_d[:])
            nc.sync.dma_start(out=tAtt[:], in_=att_d[:].to_broadcast([128, att_w]))
            nc.sync.dma_start(out=tM[:], in_=megaM_d[:])
            nc.sync.dma_start(out=tW[:], in_=megaW_d[:])

            def sl(tile_, lay, name):
                off, r, c = lay[name]
                return tile_[:r, off:off + c]

            xsT0 = sl(tA, layA, "xsT0")
            Wl1 = sl(tA, layA, "Wl1")
            xsT12 = sl(tB, layB, "xsT12") if ec > 1 else None
            xdT = sl(tB, layB, "xdT")
            Wr1 = sl(tB, layB, "Wr1")
            A1T = sl(tW, layW, "A1T")
            bias1T4 = sl(tM, layM, "bias1T4")
            wext = sl(tR, layR, "wext")
            we1b = sl(tR, layR, "we1b")
            w2ext = sl(tR, layR, "w2ext")
            we2b = sl(tR, layR, "we2b")
            G2T = sl(tG, layG, "G2T")
            att1b = tAtt[:, 0:HC]
            att2b = tAtt[:, HC:HC + C]
            Wlr2 = sl(tW, layW, "Wlr2")
            Wpv1 = sl(tW, layW, "Wpv1")
            Wout = sl(tW, layW, "Wout")
            bias2c = sl(tM, layM, "bias2col")
            bpv1c = sl(tM, layM, "bpv1col")
            boutc = sl(tM, layM, "boutcol")
            mask2 = sl(tM, layM, "mask2") if degenerate else None

            ident = cs.tile([128, 128], F32, tag="ident")
            make_identity(nc, ident[:])
            ones_row = cs.tile([1, 128], F32, tag="ones_row")
            nc.vector.memset(ones_row[:], 1.0)
            ones_col = cs.tile([128, 1], F32, tag="ones_col")
            nc.vector.memset(ones_col[:], 1.0)

            # ---- layer 1, per 128-edge chunk ----
            # engine balance: TensorE 4 bf16 matmuls; ScalarE psum-copy + exp;
            # GpSimd att-product; DVE leaky-relu, reduce, p-weighting.
            p_list, wgt_list = [], []
            for k in range(ec):
                ks = slice(k * 128, (k + 1) * 128)
                xsTk = xsT0 if k == 0 else xsT12[:, (k - 1) * 128:k * 128]
                # pb = xl + xr + e + (bl1+br1)  (emitted first: the DVE chain
                # hangs off it, while pa is only needed late, by the weighting)
                pb = psA.tile([128, HC], F32, tag="ps")
                nc.tensor.matmul(pb[:], xsTk, Wl1[:], start=True, stop=False)
                nc.tensor.matmul(pb[:], xdT[:, ks], Wr1[:], start=False, stop=False)
                nc.tensor.matmul(pb[:], wext[:, ks], we1b[:], start=False, stop=True)
                # pa = xl = x_src @ Wl1 (raw; kept in PSUM for the p-weighting)
                pa = psA.tile([128, HC], F32, tag="ps")
                nc.tensor.matmul(pa[:], xsTk, Wl1[:], start=True, stop=True)
                # m = leaky_relu(pb, 0.2) = 0.2*pb + relu(0.8*pb)
                # (relu term on ScalarE with fused scale; one DVE op from PSUM)
                r8 = wk.tile([128, HC], F32, tag="r8")
                nc.scalar.activation(r8[:], pb[:], ACT.Relu, scale=0.8)
                m = wk.tile([128, HC], F32, tag="m")
                nc.vector.scalar_tensor_tensor(
                    out=m[:], in0=pb[:], scalar=0.2, in1=r8[:],
                    op0=AL.mult, op1=AL.add)
                # logits[e,h] = sum_c m[e, h*128+c] * att1[h,c];  p = exp(logits)
                prod = wk.tile([128, HC], F32, tag="prod")
                nc.vector.tensor_mul(prod[:], m[:], att1b[:])
                logit = wk.tile([128, H], F32, tag="logit")
                nc.vector.tensor_reduce(
                    logit[:], prod[:].rearrange("e (h c) -> e h c", h=H),
                    mybir.AxisListType.X, AL.add)
                p = wk.tile([128, H], F32, tag="pf32")
                nc.scalar.activation(p[:], logit[:], ACT.Exp)
                p16 = ck.tile([128, H], F16, tag="p16")
                nc.gpsimd.tensor_copy(p16[:], p[:])
                # weighted source features: wgt = xl * p[e, h] (broadcast over c)
                wgt = ck.tile([128, HC], F16, tag="wgt")
                nc.vector.tensor_mul(
                    wgt[:].rearrange("e (h c) -> e h c", h=H),
                    pa[:].rearrange("e (h c) -> e h c", h=H),
                    bview(p[:], C))
                p_list.append(p16); wgt_list.append(wgt)

            # ---- segment denominators + aggregation via one-hot matmuls ----
            pden = psB.tile([128, H], F32, tag="pden")
            for k in range(ec):
                nc.tensor.matmul(pden[:n1, :], A1T[:, k * n1:(k + 1) * n1],
                                 p_list[k][:], start=(k == 0), stop=(k == ec - 1))
            phag = psB.tile([128, HC], F32, tag="phag")
            for k in range(ec):
                nc.tensor.matmul(phag[:n1, :], A1T[:, k * n1:(k + 1) * n1],
                                 wgt_list[k][:], start=(k == 0), stop=(k == ec - 1))
            rec = wk.tile([128, H], F32, tag="rec")
            nc.vector.tensor_scalar_add(rec[:n1, :], pden[:n1, :], 1e-16)
            nc.vector.reciprocal(rec[:n1, :], rec[:n1, :])

            # ---- per head-block: divide by denom, transpose -> hT, fused bias
            # add, elu, then immediately the layer-2 matmul for that block
            # (pipelined: block k's elu overlaps block k-1's matmul) ----
            hsb = wk.tile([128, HC], F32, tag="hsb")
            hT = wk.tile([128, H * n1], F16, tag="hT")
            pxlr2 = psC.tile([128, 2 * C], F32, tag="psc")
            for k in range(H):
                kb = slice(k * n1, (k + 1) * n1)
                hs = slice(k * 128, (k + 1) * 128)
                nc.vector.tensor_scalar_mul(
                    out=hsb[:n1, hs], in0=phag[:n1, hs],
                    scalar1=rec[:n1, k:k + 1])
                pt = psC.tile([128, 128], F32, tag="psc")
                nc.tensor.transpose(pt[:, :n1], hsb[:n1, hs],
                                    ident[:n1, :n1])
                # copy + per-partition bias in one op
                nc.vector.tensor_scalar_add(hT[:, kb], pt[:, :n1],
                                            bias1T4[:, k:k + 1])
                # elu(x) = max(x,0) + (min(exp(x),1) - 1)
                # (exp runs concurrently with the max on the other engine)
                t1 = wk.tile([128, n1], F32, tag="t1")
                nc.scalar.activation(t1[:], hT[:, kb], ACT.Exp)
                t2 = wk.tile([128, n1], F32, tag="t2")
                nc.vector.tensor_scalar(
                    out=t2[:], in0=hT[:, kb], scalar1=0.0, scalar2=-1.0,
                    op0=AL.max, op1=AL.add)
                nc.vector.scalar_tensor_tensor(
                    out=hT[:, kb], in0=t1[:], scalar=1.0, in1=t2[:],
                    op0=AL.min, op1=AL.add)
                # layer-2 node linears: one chain computes [xl2 | xr2]
                # (biases folded: bl2 -> bias2col, br2 -> we2b)
                nc.tensor.matmul(pxlr2[:n1, :], hT[:, kb],
                                 Wlr2[:, k * 256:(k + 1) * 256],
                                 start=(k == 0), stop=(k == H - 1))
            xl2 = wk.tile([128, C], F16, tag="xl2")
            nc.vector.tensor_copy(xl2[:n1, :], pxlr2[:n1, 0:C])
            xr2r = wk.tile([1, C], F32, tag="xr2r")
            nc.scalar.copy(xr2r[:], pxlr2[0:1, C:2 * C])

            # ---- layer 2 per-edge attention (single segment: dst == idx) ----
            pxle = psC.tile([128, C], F32, tag="psc")
            nc.tensor.matmul(pxle[:E2, :], G2T, xl2[:n1, :],
                             start=True, stop=True)
            xle = wk.tile([128, C], F32, tag="xle")
            nc.vector.tensor_copy(xle[:E2, :], pxle[:E2, :])
            # pm2 = e2 + (bl2+br2) + xr2[idx] + xle, accumulated on PSUM in
            # availability order so only the xle matmul is on the critical path
            pm2 = psC.tile([128, C], F32, tag="psc")
            nc.tensor.matmul(pm2[:E2, :], w2ext, we2b,
                             start=True, stop=False)
            nc.tensor.matmul(pm2[:E2, :], ones_row[:, :E2], xr2r[:],
                             start=False, stop=False)
            nc.tensor.matmul(pm2[:E2, :], G2T, xl2[:n1, :],
                             start=False, stop=True)
            # m2 = leaky_relu(pm2, 0.2) = 0.2*pm2 + relu(0.8*pm2)
            r82 = wk.tile([128, C], F32, tag="r82")
            nc.scalar.activation(r82[:E2, :], pm2[:E2, :], ACT.Relu, scale=0.8)
            m2 = wk.tile([128, C], F32, tag="m2")
            nc.vector.scalar_tensor_tensor(
                out=m2[:E2, :], in0=pm2[:E2, :], scalar=0.2, in1=r82[:E2, :],
                op0=AL.mult, op1=AL.add)
            prod2 = wk.tile([128, C], F32, tag="prod2")
            nc.vector.tensor_mul(prod2[:E2, :], m2[:E2, :], att2b[:E2, :])
            logit2 = wk.tile([128, 1], F32, tag="logit2")
            nc.vector.tensor_reduce(logit2[:E2, :], prod2[:E2, :],
                                    mybir.AxisListType.X, AL.add)
            p2 = wk.tile([128, 1], F32, tag="p2")
            nc.scalar.activation(p2[:E2, :], logit2[:E2, :], ACT.Exp)
            if degenerate:
                nc.vector.tensor_mul(p2[:E2, :], p2[:E2, :], mask2)
            # zT_raw = xle^T @ p2 ; denom broadcast back via K=1 matmul
            pd2 = psC.tile([1, 1], F32, tag="psc")
            nc.tensor.matmul(pd2[:, :], p2[:E2, :], ones_col[:E2, :],
                             start=True, stop=True)
            pzr = psC.tile([128, 1], F32, tag="psc")
            nc.tensor.matmul(pzr[:, :], xle[:E2, :], p2[:E2, :],
                             start=True, stop=True)
            d2 = wk.tile([1, 1], F32, tag="d2")
            if degenerate:
                # masked-out edges can make denom exactly 0; keep the epsilon
                nc.vector.tensor_scalar_add(d2[:], pd2[:], 1e-16)
                nc.vector.reciprocal(d2[:], d2[:])
            else:
                # denom >= exp(min logit) >> 1e-16: epsilon is a strict no-op
                nc.vector.reciprocal(d2[:], pd2[:])
            prb = psC.tile([128, 1], F32, tag="psc")
            nc.tensor.matmul(prb[:, :], ones_row[:], d2[:], start=True, stop=True)
            rb = wk.tile([128, 1], F32, tag="rb")
            nc.vector.tensor_copy(rb[:], prb[:])
            zT = wk.tile([128, 1], F16, tag="zT")
            nc.vector.scalar_tensor_tensor(
                out=zT[:], in0=pzr[:], scalar=rb[:], in1=bias2c,
                op0=AL.mult, op1=AL.add)

            # ---- actor/critic heads, fully in transposed layout ----
            ph = psC.tile([128, 1], F32, tag="psc")
            nc.tensor.matmul(ph[:, :], Wpv1, zT[:], start=True, stop=True)
            hidT = wk.tile([128, 1], F16, tag="hidT")
            nc.vector.tensor_scalar(
                out=hidT[:], in0=ph[:], scalar1=bpv1c, scalar2=0.0,
                op0=AL.add, op1=AL.max)
            po = psC.tile([3, 1], F32, tag="psc")
            nc.tensor.matmul(po[:, :], Wout, hidT[:], start=True, stop=True)
            osb = wk.tile([3, 1], F32, tag="osb")
            nc.vector.tensor_scalar(
                out=osb[:], in0=po[:], scalar1=boutc, scalar2=None,
                op0=AL.add)
            nc.sync.dma_start(out=out_d[:], in_=osb[:])
    return nc


def _prepare(inputs):
    """Host-side exact pruning + operand layout.  Returns (dev_inputs, dims, lays)."""
    x = np.asarray(inputs["x"], np.float32)
    ei = np.asarray(inputs["edge_index"]).astype(np.int64)
    ew = np.asarray(inputs["edge_weight"], np.float32).reshape(-1)
    idx = int(np.asarray(inputs["node_to_assign_idx"]))
    src, dst = ei[0], ei[1]
    n_nodes = x.shape[0]

    e2_mask = dst == idx
    src2 = src[e2_mask]
    w2 = ew[e2_mask]
    E2 = int(src2.shape[0])
    degenerate = E2 == 0
    if degenerate:  # keep shapes >=1; contribution masked to zero on device
        src2 = np.array([idx]); w2 = np.zeros(1, np.float32)
        E2 = 1

    rest = np.unique(src2)
    rest = rest[rest != idx]
    S1 = np.concatenate([np.array([idx], np.int64), rest.astype(np.int64)])
    n1 = int(S1.shape[0])

    in_S1 = np.zeros(n_nodes, bool)
    in_S1[S1] = True
    e1_mask = in_S1[dst]
    src1, dst1, w1 = src[e1_mask], dst[e1_mask], ew[e1_mask]
    E1 = int(src1.shape[0])
    E1p = max(128, ((E1 + 127) // 128) * 128)
    ec = E1p // 128

    pos1 = np.full(n_nodes, -1, np.int64)
    pos1[S1] = np.arange(n1)

    xsT = np.zeros((128, E1p), np.float32)
    xsT[:, :E1] = x[src1].T
    xdT = np.zeros((128, E1p), np.float32)
    xdT[:, :E1] = x[dst1].T
    wext = np.zeros((2, E1p), np.float32)
    wext[0, :] = 1.0
    wext[1, :E1] = w1

    A1T = np.zeros((128, ec * n1), np.float32)
    e_ids = np.arange(E1)
    A1T[e_ids % 128, (e_ids // 128) * n1 + pos1[dst1]] = 1.0

    G2T = np.zeros((n1, E2), np.float32)
    G2T[pos1[src2], np.arange(E2)] = 1.0
    w2ext = np.stack([np.ones(E2, np.float32), w2.astype(np.float32)])

    g = lambda k: np.asarray(inputs[k], np.float32)
    # [128, H, 2C]: per k-block the columns are [Wl2 block | Wr2 block]
    Wlr2 = np.concatenate(
        [g("Wl2").reshape(H, 128, C).transpose(1, 0, 2),
         g("Wr2").reshape(H, 128, C).transpose(1, 0, 2)],
        axis=2).reshape(128, H * 2 * C)

    import ml_dtypes
    l1dt = ml_dtypes.bfloat16 if BF16_L1 else np.float32
    pa = _Packer(128, l1dt)
    pa.add("xsT0", xsT[:, :128])
    pa.add("Wl1", g("Wl1"))
    pb = _Packer(128, l1dt)
    if ec > 1:
        pb.add("xsT12", xsT[:, 128:])
    pb.add("xdT", xdT)
    pb.add("Wr1", g("Wr1"))
    pr = _Packer(2, l1dt)
    pr.add("wext", wext)
    pr.add("we1b", np.stack([g("bl1") + g("br1"), g("We1").reshape(-1)]))
    pr.add("w2ext", w2ext)
    pr.add("we2b", np.stack([g("bl2") + g("br2"), g("We2").reshape(-1)]))
    pg = _Packer(n1, l1dt)
    pg.add("G2T", G2T)
    pm = _Packer(128)
    pm.add("bias1T4", np.ascontiguousarray((g("bias1") + g("bl1")).reshape(H, 128).T))
    pm.add("bias2col", (g("bias2") + g("bl2")).reshape(128, 1))
    pm.add("bpv1col", np.concatenate([g("bp1"), g("bv1")]).reshape(128, 1))
    pm.add("boutcol", np.concatenate([g("bp2"), g("bv2")]).reshape(3, 1))
    if degenerate:
        pm.add("mask2", np.zeros((E2, 1), np.float32))
    pw = _Packer(128, l1dt)
    pw.add("A1T", A1T)
    pw.add("Wlr2", Wlr2)
    pw.add("Wpv1", np.concatenate([g("Wp1"), g("Wv1")], axis=1))
    pw.add("Wout", np.concatenate(
        [np.concatenate([g("Wp2"), np.zeros((MLP, 1), np.float32)], axis=1),
         np.concatenate([np.zeros((MLP, 2), np.float32), g("Wv2")], axis=1)]))

    attrow = np.concatenate(
        [g("att1").reshape(-1), g("att2").reshape(-1)]).reshape(1, HC + C)

    megaA, layA = pa.finalize()
    megaB, layB = pb.finalize()
    megaR, layR = pr.finalize()
    megaG, layG = pg.finalize()
    megaM, layM = pm.finalize()
    megaW, layW = pw.finalize()
    dev = {"megaA": megaA, "megaB": megaB, "megaR": megaR, "megaG": megaG,
           "megaM": megaM, "megaW": megaW, "attrow": attrow}
    return (dev, (E1p, ec, n1, E2, degenerate),
            (layA, layB, layR, layG, layM, layW, HC + C))


def _numpy_fallback(inputs):
    """Exact reference math in numpy (used only if the subgraph exceeds the
    single-tile device layout, which cannot happen for the problem's data)."""
    x = np.asarray(inputs["x"], np.float32)
    ei = np.asarray(inputs["edge_index"]).astype(np.int64)
    ew = np.asarray(inputs["edge_weight"], np.float32)
    idx = int(np.asarray(inputs["node_to_assign_idx"]))
    src, dst = ei[0], ei[1]
    n = x.shape[0]
    g = lambda k: np.asarray(inputs[k], np.float32)

    def layer(xf, Wl, bl, Wr, br, We, att, bias, heads, ch, concat):
        xl = (xf @ Wl + bl).reshape(-1, heads, ch)
        xr = (xf @ Wr + br).reshape(-1, heads, ch)
        e = (ew @ We).reshape(-1, heads, ch)
        m = xl[src] + xr[dst] + e
        m = np.where(m > 0, m, 0.2 * m)
        logits = np.einsum("ehc,hc->eh", m, att.reshape(heads, ch))
        amax = np.full((n, heads), -np.inf, np.float32)
        np.maximum.at(amax, dst, logits)
        amax = np.where(np.isfinite(amax), amax, 0.0)
        p = np.exp(logits - amax[dst])
        den = np.zeros((n, heads), np.float32)
        np.add.at(den, dst, p)
        alpha = p / (den[dst] + 1e-16)
        out = np.zeros((n, heads, ch), np.float32)
        np.add.at(out, dst, xl[src] * alpha[..., None])
        out = out.reshape(n, heads * ch) if concat else out.mean(1)
        return out + bias

    h = layer(x, g("Wl1"), g("bl1"), g("Wr1"), g("br1"), g("We1"), g("att1"),
              g("bias1"), H, C, True)
    h = np.where(h > 0, h, np.exp(np.minimum(h, 0)) - 1)
    emb = layer(h, g("Wl2"), g("bl2"), g("Wr2"), g("br2"), g("We2"), g("att2"),
                g("bias2"), 1, C, False)
    z = emb[idx]
    a = np.maximum(z @ g("Wp1") + g("bp1"), 0) @ g("Wp2") + g("bp2")
    v = np.maximum(z @ g("Wv1") + g("bv1"), 0) @ g("Wv2") + g("bv2")
    return a.astype(np.float32), v.astype(np.float32)


def kernel(**inputs):
    dev, dims, lays = _prepare(inputs)
    E1p, ec, n1, E2, degenerate = dims
    if n1 > 128 or E2 > 128:
        return _numpy_fallback(inputs)

    import concourse.bacc as bacc
    from concourse.bass_utils import run_bass_kernel_spmd

    nc = bacc.Bacc("TRN2", target_bir_lowering=False, debug=False)
    _build(nc, dims, lays)
    nc.compile()
    res = run_bass_kernel_spmd(nc, [dict(dev) for _ in range(8)], list(range(8)))
    out = np.asarray(res.results[0]["out"], np.float32).reshape(3)
    return out[:2].copy(), out[2:3].copy()


# revision 73
# speedup vs baseline: 1.0314x; 1.0208x over previous
"""Trainium2 Bass kernel for nn_ActorCriticGAT (2-layer GATv2 + actor/critic heads).

The reference network's output is (action_logits[2], state_value[1]), both computed
solely from emb[node_to_assign_idx].  GATv2 layers couple nodes only through
per-destination segment softmax / segment sum over in-edges, so the output depends
exactly on the 2-hop in-neighborhood of node_to_assign_idx:

  layer-2 edges  E2 = { e : dst[e] == idx }              (~17 edges)
  needed h nodes S1 = {idx} U src[E2]                    (~18 nodes)
  layer-1 edges  E1 = { e : dst[e] in S1 }               (~310 edges)

Host side (numpy) does only index work — boolean masks, gathers of x rows, and
one-hot scatter/gather matrices (the same work a DMA engine's descriptors would
do).  All model arithmetic — the linear layers, leaky-relu, per-segment softmax,
alpha-weighted aggregation, elu, and both MLP heads — runs on the NeuronCores as
TensorE matmuls + Vector/Scalar engine elementwise ops.  Segment softmax/scatter
are expressed as one-hot matmuls (A1^T @ .), so no indirect addressing is needed
on-device.

All device constants are packed host-side into a handful of contiguous blocks,
grouped by row count and by when the compute needs them, and loaded with one
dma_start each (large transfers parallelize across all 16 SDMA engines; many
small DMAs each pay the ~2us fixed completion cost).  The pack feeding the
first matmul chain is shipped first so TensorE starts as early as possible.

Matmul inputs are bf16 (PSUM accumulation and all Vector/Scalar-engine math
stay fp32) — this halves both the weight-DMA bytes and the TensorE cost and
costs ~1.5e-3 relative error against the fp32 reference, far inside the 2e-2
accuracy gate.

Numerical notes (exact or negligible vs the reference):
 - exp() without max-subtraction: logits here are O(1), and alpha is the same
   rational function of the logits either way (the 1e-16 epsilon shifts by
   exp(-amax), which is negligible at these magnitudes).
 - division uses (denom + 1e-16), matching the reference formula exactly
   (dropped for the layer-2 softmax when real in-edges exist: denom >= exp(min
   logit) >> 1e-16 makes it a strict no-op there).
 - leaky_relu(x) = 0.2x + relu(0.8x) and elu(x) = max(x,0) + min(exp(x),1) - 1
   are exact rewrites chosen for engine balance.
 - linear-layer biases that feed the alpha-weighted aggregation are applied
   after aggregation (bias * sum(alpha) == bias up to the 1e-16 epsilon); for
   this model every such bias is exactly zero anyway.

The work is replicated SPMD across all 8 NeuronCores (the pruned subgraph is far
below one core's granularity, so partitioning it would only add collective
latency); core 0's output is returned.
"""
import numpy as np

N_NODES = 50000
D = 128          # input feature dim
C = 128          # channels per head
H = 4            # heads, layer 1
HC = H * C       # 512
MLP = 64


BF16_L1 = True   # layer-1 edge matmuls in bf16 (PSUM accumulation stays fp32)


class _Packer:
    """Pack same-row-count tensors side by side into one [rows, W] block so a
    single dma_start ships them (large DMAs parallelize across all 16 SDMA
    engines; many small DMAs each pay the ~2us fixed completion cost)."""

    def __init__(self, rows, dtype=np.float32):
        self.rows = rows
        self.dtype = dtype
        self.entries = {}
        self.w = 0

    def add(self, name, arr):
        r, c = arr.shape
        assert r <= self.rows
        self.entries[name] = (self.w, r, c, arr)
        self.w += c

    def finalize(self):
        out = np.zeros((self.rows, self.w), np.float32)
        lay = {}
        for name, (off, r, c, arr) in self.entries.items():
            out[:r, off:off + c] = arr
            lay[name] = (off, r, c)
        return out.astype(self.dtype), lay


def _build(nc, dims, lays):
    """Build the Bass/Tile graph.  dims = (E1p, ec, n1, E2, degenerate)."""
    import concourse.bass as bass
    import concourse.tile as tile
    from concourse import mybir
    from concourse.masks import make_identity

    F32 = mybir.dt.float32
    F16 = mybir.dt.bfloat16
    L1DT = F16 if BF16_L1 else F32
    E1p, ec, n1, E2, degenerate = dims
    layA, layB, layR, layG, layM, layW, att_w = lays

    def mega_param(name, rows, lay, dt=F32):
        w = max(o + c for o, _, c in lay.values())
        return nc.declare_dram_parameter(name, [rows, w], dt, isOutput=False)

    megaA_d = mega_param("megaA", 128, layA, L1DT)
    megaB_d = mega_param("megaB", 128, layB, L1DT)
    megaR_d = mega_param("megaR", 2, layR, L1DT)
    megaG_d = mega_param("megaG", n1, layG, L1DT)
    megaM_d = mega_param("megaM", 128, layM)
    megaW_d = mega_param("megaW", 128, layW, L1DT)
    att_d = nc.declare_dram_parameter("attrow", [1, att_w], F32, isOutput=False)
    out_d = nc.declare_dram_parameter("out", [3, 1], F32, isOutput=True)

    AL = mybir.AluOpType
    ACT = mybir.ActivationFunctionType

    def bview(ap, inner):
        """Append a stride-0 inner free dim (broadcast) to an AP."""
        return bass.AP(tensor=ap.tensor, offset=ap.offset, ap=[*ap.ap, [0, inner]])

    with tile.TileContext(nc) as tc:
        with (
            tc.tile_pool(name="const", bufs=1) as cs,
            tc.tile_pool(name="work", bufs=2) as wk,
            tc.tile_pool(name="chunk", bufs=max(ec, 1)) as ck,
            tc.tile_pool(name="psA", bufs=4, space="PSUM") as psA,
            tc.tile_pool(name="psB", bufs=1, space="PSUM") as psB,
            tc.tile_pool(name="psC", bufs=2, space="PSUM") as psC,
        ):
            # warm the ScalarE activation table (Exp) while DMAs run
            warm = cs.tile([1, 1], F32, tag="warm")
            nc.vector.memset(warm[:], 0.0)
            nc.scalar.activation(warm[:], warm[:], ACT.Exp)

            tA = cs.tile([128, megaA_d.shape[1]], L1DT, tag="tA")
            tB = cs.tile([128, megaB_d.shape[1]], L1DT, tag="tB")
            tR = cs.tile([2, megaR_d.shape[1]], L1DT, tag="tR")
            tG = cs.tile([n1, megaG_d.shape[1]], L1DT, tag="tG")
            tM = cs.tile([128, megaM_d.shape[1]], F32, tag="tM")
            tW = cs.tile([128, megaW_d.shape[1]], L1DT, tag="tW")
            tAtt = cs.tile([128, att_w], F32, tag="tAtt")
            nc.sync.dma_start(out=tR[:], in_=megaR_d[:])
            nc.sync.dma_start(out=tA[:], in_=megaA_d[:])
            nc.sync.dma_start(out=tB[:], in_=megaB_d[:])
            nc.sync.dma_start(out=tG[:], in_=megaG_d[:])
            nc.sync.dma_start(out=tAtt[:], in_=att_d[:].to_broadcast([128, att_w]))
            nc.sync.dma_start(out=tM[:], in_=megaM_d[:])
            nc.sync.dma_start(out=tW[:], in_=megaW_d[:])

            def sl(tile_, lay, name):
                off, r, c = lay[name]
                return tile_[:r, off:off + c]

            xsT0 = sl(tA, layA, "xsT0")
            Wl1 = sl(tA, layA, "Wl1")
            W1f = sl(tA, layA, "W1f")
            xsT12 = sl(tB, layB, "xsT12") if ec > 1 else None
            xdT = sl(tB, layB, "xdT")
            Wr1 = sl(tB, layB, "Wr1")
            Wr1f = sl(tB, layB, "Wr1f")
            we1bf = sl(tR, layR, "we1bf")
            A1T = sl(tW, layW, "A1T")
            bias1T4 = sl(tM, layM, "bias1T4")
            wext = sl(tR, layR, "wext")
            we1b = sl(tR, layR, "we1b")
            w2ext = sl(tR, layR, "w2ext")
            we2b = sl(tR, layR, "we2b")
            G2T = sl(tG, layG, "G2T")
            att1b = tAtt[:, 0:HC]
            att2b = tAtt[:, HC:HC + C]
            Wlr2 = sl(tW, layW, "Wlr2")
            Wpv1 = sl(tW, layW, "Wpv1")
            Wout = sl(tW, layW, "Wout")
            bias2c = sl(tM, layM, "bias2col")
            bpv1c = sl(tM, layM, "bpv1col")
            boutc = sl(tM, layM, "boutcol")
            mask2 = sl(tM, layM, "mask2") if degenerate else None

            ident = cs.tile([128, 128], F32, tag="ident")
            make_identity(nc, ident[:])
            ones_row = cs.tile([1, 128], F32, tag="ones_row")
            nc.vector.memset(ones_row[:], 1.0)
            ones_col = cs.tile([128, 1], F32, tag="ones_col")
            nc.vector.memset(ones_col[:], 1.0)

            # ---- layer 1, per 128-edge chunk ----
            # engine balance: TensorE 4 bf16 matmuls; ScalarE psum-copy + exp;
            # GpSimd att-product; DVE leaky-relu, reduce, p-weighting.
            p_list, wgt_list = [], []
            for k in range(ec):
                ks = slice(k * 128, (k + 1) * 128)
                xsTk = xsT0 if k == 0 else xsT12[:, (k - 1) * 128:k * 128]
                # pb = xl + xr + e + (bl1+br1)  (emitted first: the DVE chain
                # hangs off it, while pa is only needed late, by the weighting)
                pb = psA.tile([128, HC], F32, tag="ps")
                nc.tensor.matmul(pb[:], xsTk, Wl1[:], start=True, stop=False)
                nc.tensor.matmul(pb[:], xdT[:, ks], Wr1[:], start=False, stop=False)
                nc.tensor.matmul(pb[:], wext[:, ks], we1b[:], start=False, stop=True)
                # pa = xl = x_src @ Wl1 (raw; kept in PSUM for the p-weighting)
                pa = psA.tile([128, HC], F32, tag="ps")
                nc.tensor.matmul(pa[:], xsTk, Wl1[:], start=True, stop=True)
                # logits[e,h] = sum_c leaky(s)[e,hc]*att1[h,c] with
                # leaky(s) = 0.2*s + relu(0.8*s) split into
                #   linear term:  0.2*sum(att*s)  -> TensorE, via host-folded
                #                 weights Wf = 0.2*(W @ att_h)   [128, H]
                #   relu term:    sum(att * relu(0.8*s))  -> ScalarE + DVE
                pt1 = psA.tile([128, H], F32, tag="ps")
                nc.tensor.matmul(pt1[:], xsTk, W1f, start=True, stop=False)
                nc.tensor.matmul(pt1[:], xdT[:, ks], Wr1f, start=False, stop=False)
                nc.tensor.matmul(pt1[:], wext[:, ks], we1bf, start=False, stop=True)
                r8 = wk.tile([128, HC], F32, tag="r8")
                nc.scalar.activation(r8[:], pb[:], ACT.Relu, scale=0.8)
                prod = wk.tile([128, HC], F32, tag="prod")
                nc.vector.tensor_mul(prod[:], r8[:], att1b[:])
                red = wk.tile([128, H], F32, tag="red")
                nc.vector.tensor_reduce(
                    red[:], prod[:].rearrange("e (h c) -> e h c", h=H),
                    mybir.AxisListType.X, AL.add)
                logit = wk.tile([128, H], F32, tag="logit")
                nc.vector.tensor_add(logit[:], red[:], pt1[:])
                p = wk.tile([128, H], F32, tag="pf32")
                nc.scalar.activation(p[:], logit[:], ACT.Exp)
                p16 = ck.tile([128, H], F16, tag="p16")
                nc.gpsimd.tensor_copy(p16[:], p[:])
                # weighted source features: wgt = xl * p[e, h] (broadcast over c)
                wgt = ck.tile([128, HC], F16, tag="wgt")
                nc.vector.tensor_mul(
                    wgt[:].rearrange("e (h c) -> e h c", h=H),
                    pa[:].rearrange("e (h c) -> e h c", h=H),
                    bview(p[:], C))
                p_list.append(p16); wgt_list.append(wgt)

            # ---- segment denominators + aggregation via one-hot matmuls ----
            pden = psB.tile([128, H], F32, tag="pden")
            for k in range(ec):
                nc.tensor.matmul(pden[:n1, :], A1T[:, k * n1:(k + 1) * n1],
                                 p_list[k][:], start=(k == 0), stop=(k == ec - 1))
            phag = psB.tile([128, HC], F32, tag="phag")
            for k in range(ec):
                nc.tensor.matmul(phag[:n1, :], A1T[:, k * n1:(k + 1) * n1],
                                 wgt_list[k][:], start=(k == 0), stop=(k == ec - 1))
            rec = wk.tile([128, H], F32, tag="rec")
            nc.vector.tensor_scalar_add(rec[:n1, :], pden[:n1, :], 1e-16)
            nc.vector.reciprocal(rec[:n1, :], rec[:n1, :])

            # ---- per head-block: divide by denom, transpose -> hT, fused bias
            # add, elu, then immediately the layer-2 matmul for that block
            # (pipelined: block k's elu overlaps block k-1's matmul) ----
            hsb = wk.tile([128, HC], F32, tag="hsb")
            hT = wk.tile([128, H * n1], F16, tag="hT")
            pxlr2 = psC.tile([128, 2 * C], F32, tag="psc")
            for k in range(H):
                kb = slice(k * n1, (k + 1) * n1)
                hs = slice(k * 128, (k + 1) * 128)
                nc.vector.tensor_scalar_mul(
                    out=hsb[:n1, hs], in0=phag[:n1, hs],
                    scalar1=rec[:n1, k:k + 1])
                pt = psC.tile([128, 128], F32, tag="psc")
                nc.tensor.transpose(pt[:, :n1], hsb[:n1, hs],
                                    ident[:n1, :n1])
                # copy + per-partition bias in one op
                nc.vector.tensor_scalar_add(hT[:, kb], pt[:, :n1],
                                            bias1T4[:, k:k + 1])
                # elu(x) = max(x,0) + (min(exp(x),1) - 1)
                # (exp runs concurrently with the max on the other engine)
                t1 = wk.tile([128, n1], F32, tag="t1")
                nc.scalar.activation(t1[:], hT[:, kb], ACT.Exp)
                t2 = wk.tile([128, n1], F32, tag="t2")
                nc.vector.tensor_scalar(
                    out=t2[:], in0=hT[:, kb], scalar1=0.0, scalar2=-1.0,
                    op0=AL.max, op1=AL.add)
                nc.vector.scalar_tensor_tensor(
                    out=hT[:, kb], in0=t1[:], scalar=1.0, in1=t2[:],
                    op0=AL.min, op1=AL.add)
                # layer-2 node linears: one chain computes [xl2 | xr2]
                # (biases folded: bl2 -> bias2col, br2 -> we2b)
                nc.tensor.matmul(pxlr2[:n1, :], hT[:, kb],
                                 Wlr2[:, k * 256:(k + 1) * 256],
                                 start=(k == 0), stop=(k == H - 1))
            xl2 = wk.tile([128, C], F16, tag="xl2")
            nc.vector.tensor_copy(xl2[:n1, :], pxlr2[:n1, 0:C])
            xr2r = wk.tile([1, C], F32, tag="xr2r")
            nc.scalar.copy(xr2r[:], pxlr2[0:1, C:2 * C])

            # ---- layer 2 per-edge attention (single segment: dst == idx) ----
            pxle = psC.tile([128, C], F32, tag="psc")
            nc.tensor.matmul(pxle[:E2, :], G2T, xl2[:n1, :],
                             start=True, stop=True)
            xle = wk.tile([128, C], F32, tag="xle")
            nc.vector.tensor_copy(xle[:E2, :], pxle[:E2, :])
            # pm2 = e2 + (bl2+br2) + xr2[idx] + xle, accumulated on PSUM in
            # availability order so only the xle matmul is on the critical path
            pm2 = psC.tile([128, C], F32, tag="psc")
            nc.tensor.matmul(pm2[:E2, :], w2ext, we2b,
                             start=True, stop=False)
            nc.tensor.matmul(pm2[:E2, :], ones_row[:, :E2], xr2r[:],
                             start=False, stop=False)
            nc.tensor.matmul(pm2[:E2, :], G2T, xl2[:n1, :],
                             start=False, stop=True)
            # m2 = leaky_relu(pm2, 0.2) = 0.2*pm2 + relu(0.8*pm2)
            r82 = wk.tile([128, C], F32, tag="r82")
            nc.scalar.activation(r82[:E2, :], pm2[:E2, :], ACT.Relu, scale=0.8)
            m2 = wk.tile([128, C], F32, tag="m2")
            nc.vector.scalar_tensor_tensor(
                out=m2[:E2, :], in0=pm2[:E2, :], scalar=0.2, in1=r82[:E2, :],
                op0=AL.mult, op1=AL.add)
            prod2 = wk.tile([128, C], F32, tag="prod2")
            nc.vector.tensor_mul(prod2[:E2, :], m2[:E2, :], att2b[:E2, :])
            logit2 = wk.tile([128, 1], F32, tag="logit2")
            nc.vector.tensor_reduce(logit2[:E2, :], prod2[:E2, :],
                                    mybir.AxisListType.X, AL.add)
            p2 = wk.tile([128, 1], F32, tag="p2")
            nc.scalar.activation(p2[:E2, :], logit2[:E2, :], ACT.Exp)
            if degenerate:
                nc.vector.tensor_mul(p2[:E2, :], p2[:E2, :], mask2)
            # zT_raw = xle^T @ p2 ; denom broadcast back via K=1 matmul
            pd2 = psC.tile([1, 1], F32, tag="psc")
            nc.tensor.matmul(pd2[:, :], p2[:E2, :], ones_col[:E2, :],
                             start=True, stop=True)
            pzr = psC.tile([128, 1], F32, tag="psc")
            nc.tensor.matmul(pzr[:, :], xle[:E2, :], p2[:E2, :],
                             start=True, stop=True)
            d2 = wk.tile([1, 1], F32, tag="d2")
            if degenerate:
                # masked-out edges can make denom exactly 0; keep the epsilon
                nc.vector.tensor_scalar_add(d2[:], pd2[:], 1e-16)
                nc.vector.reciprocal(d2[:], d2[:])
            else:
                # denom >= exp(min logit) >> 1e-16: epsilon is a strict no-op
                nc.vector.reciprocal(d2[:], pd2[:])
            prb = psC.tile([128, 1], F32, tag="psc")
            nc.tensor.matmul(prb[:, :], ones_row[:], d2[:], start=True, stop=True)
            rb = wk.tile([128, 1], F32, tag="rb")
            nc.vector.tensor_copy(rb[:], prb[:])
            zT = wk.tile([128, 1], F16, tag="zT")
            nc.vector.scalar_tensor_tensor(
                out=zT[:], in0=pzr[:], scalar=rb[:], in1=bias2c,
                op0=AL.mult, op1=AL.add)

            # ---- actor/critic heads, fully in transposed layout ----
            ph = psC.tile([128, 1], F32, tag="psc")
            nc.tensor.matmul(ph[:, :], Wpv1, zT[:], start=True, stop=True)
            hidT = wk.tile([128, 1], F16, tag="hidT")
            nc.vector.tensor_scalar(
                out=hidT[:], in0=ph[:], scalar1=bpv1c, scalar2=0.0,
                op0=AL.add, op1=AL.max)
            po = psC.tile([3, 1], F32, tag="psc")
            nc.tensor.matmul(po[:, :], Wout, hidT[:], start=True, stop=True)
            osb = wk.tile([3, 1], F32, tag="osb")
            nc.vector.tensor_scalar(
                out=osb[:], in0=po[:], scalar1=boutc, scalar2=None,
                op0=AL.add)
            nc.sync.dma_start(out=out_d[:], in_=osb[:])
    return nc


def _prepare(inputs):
    """Host-side exact pruning + operand layout.  Returns (dev_inputs, dims, lays)."""
    x = np.asarray(inputs["x"], np.float32)
    ei = np.asarray(inputs["edge_index"]).astype(np.int64)
    ew = np.asarray(inputs["edge_weight"], np.float32).reshape(-1)
    idx = int(np.asarray(inputs["node_to_assign_idx"]))
    src, dst = ei[0], ei[1]
    n_nodes = x.shape[0]

    e2_mask = dst == idx
    src2 = src[e2_mask]
    w2 = ew[e2_mask]
    E2 = int(src2.shape[0])
    degenerate = E2 == 0
    if degenerate:  # keep shapes >=1; contribution masked to zero on device
        src2 = np.array([idx]); w2 = np.zeros(1, np.float32)
        E2 = 1

    rest = np.unique(src2)
    rest = rest[rest != idx]
    S1 = np.concatenate([np.array([idx], np.int64), rest.astype(np.int64)])
    n1 = int(S1.shape[0])

    in_S1 = np.zeros(n_nodes, bool)
    in_S1[S1] = True
    e1_mask = in_S1[dst]
    src1, dst1, w1 = src[e1_mask], dst[e1_mask], ew[e1_mask]
    E1 = int(src1.shape[0])
    E1p = max(128, ((E1 + 127) // 128) * 128)
    ec = E1p // 128

    pos1 = np.full(n_nodes, -1, np.int64)
    pos1[S1] = np.arange(n1)

    xsT = np.zeros((128, E1p), np.float32)
    xsT[:, :E1] = x[src1].T
    xdT = np.zeros((128, E1p), np.float32)
    xdT[:, :E1] = x[dst1].T
    wext = np.zeros((2, E1p), np.float32)
    wext[0, :] = 1.0
    wext[1, :E1] = w1

    A1T = np.zeros((128, ec * n1), np.float32)
    e_ids = np.arange(E1)
    A1T[e_ids % 128, (e_ids // 128) * n1 + pos1[dst1]] = 1.0

    G2T = np.zeros((n1, E2), np.float32)
    G2T[pos1[src2], np.arange(E2)] = 1.0
    w2ext = np.stack([np.ones(E2, np.float32), w2.astype(np.float32)])

    g = lambda k: np.asarray(inputs[k], np.float32)
    # [128, H, 2C]: per k-block the columns are [Wl2 block | Wr2 block]
    Wlr2 = np.concatenate(
        [g("Wl2").reshape(H, 128, C).transpose(1, 0, 2),
         g("Wr2").reshape(H, 128, C).transpose(1, 0, 2)],
        axis=2).reshape(128, H * 2 * C)

    import ml_dtypes
    l1dt = ml_dtypes.bfloat16 if BF16_L1 else np.float32
    # host-folded attention weights: Wf[d,h] = 0.2 * sum_c W[d,h*C+c]*att1[h,c]
    att1 = g("att1").reshape(H, C)
    W1f = 0.2 * np.einsum("dhc,hc->dh", g("Wl1").reshape(D, H, C), att1)
    Wr1f = 0.2 * np.einsum("dhc,hc->dh", g("Wr1").reshape(D, H, C), att1)
    b1f = 0.2 * np.einsum("hc,hc->h", (g("bl1") + g("br1")).reshape(H, C), att1)
    We1f = 0.2 * np.einsum("hc,hc->h", g("We1").reshape(H, C), att1)

    pa = _Packer(128, l1dt)
    pa.add("xsT0", xsT[:, :128])
    pa.add("Wl1", g("Wl1"))
    pa.add("W1f", W1f)
    pb = _Packer(128, l1dt)
    if ec > 1:
        pb.add("xsT12", xsT[:, 128:])
    pb.add("xdT", xdT)
    pb.add("Wr1", g("Wr1"))
    pb.add("Wr1f", Wr1f)
    pr = _Packer(2, l1dt)
    pr.add("wext", wext)
    pr.add("we1b", np.stack([g("bl1") + g("br1"), g("We1").reshape(-1)]))
    pr.add("we1bf", np.stack([b1f, We1f]))
    pr.add("w2ext", w2ext)
    pr.add("we2b", np.stack([g("bl2") + g("br2"), g("We2").reshape(-1)]))
    pg = _Packer(n1, l1dt)
    pg.add("G2T", G2T)
    pm = _Packer(128)
    pm.add("bias1T4", np.ascontiguousarray((g("bias1") + g("bl1")).reshape(H, 128).T))
    pm.add("bias2col", (g("bias2") + g("bl2")).reshape(128, 1))
    pm.add("bpv1col", np.concatenate([g("bp1"), g("bv1")]).reshape(128, 1))
    pm.add("boutcol", np.concatenate([g("bp2"), g("bv2")]).reshape(3, 1))
    if degenerate:
        pm.add("mask2", np.zeros((E2, 1), np.float32))
    pw = _Packer(128, l1dt)
    pw.add("A1T", A1T)
    pw.add("Wlr2", Wlr2)
    pw.add("Wpv1", np.concatenate([g("Wp1"), g("Wv1")], axis=1))
    pw.add("Wout", np.concatenate(
        [np.concatenate([g("Wp2"), np.zeros((MLP, 1), np.float32)], axis=1),
         np.concatenate([np.zeros((MLP, 2), np.float32), g("Wv2")], axis=1)]))

    attrow = np.concatenate(
        [g("att1").reshape(-1), g("att2").reshape(-1)]).reshape(1, HC + C)

    megaA, layA = pa.finalize()
    megaB, layB = pb.finalize()
    megaR, layR = pr.finalize()
    megaG, layG = pg.finalize()
    megaM, layM = pm.finalize()
    megaW, layW = pw.finalize()
    dev = {"megaA": megaA, "megaB": megaB, "megaR": megaR, "megaG": megaG,
           "megaM": megaM, "megaW": megaW, "attrow": attrow}
    return (dev, (E1p, ec, n1, E2, degenerate),
            (layA, layB, layR, layG, layM, layW, HC + C))


def _numpy_fallback(inputs):
    """Exact reference math in numpy (used only if the subgraph exceeds the
    single-tile device layout, which cannot happen for the problem's data)."""
    x = np.asarray(inputs["x"], np.float32)
    ei = np.asarray(inputs["edge_index"]).astype(np.int64)
    ew = np.asarray(inputs["edge_weight"], np.float32)
    idx = int(np.asarray(inputs["node_to_assign_idx"]))
    src, dst = ei[0], ei[1]
    n = x.shape[0]
    g = lambda k: np.asarray(inputs[k], np.float32)

    def layer(xf, Wl, bl, Wr, br, We, att, bias, heads, ch, concat):
        xl = (xf @ Wl + bl).reshape(-1, heads, ch)
        xr = (xf @ Wr + br).reshape(-1, heads, ch)
        e = (ew @ We).reshape(-1, heads, ch)
        m = xl[src] + xr[dst] + e
        m = np.where(m > 0, m, 0.2 * m)
        logits = np.einsum("ehc,hc->eh", m, att.reshape(heads, ch))
        amax = np.full((n, heads), -np.inf, np.float32)
        np.maximum.at(amax, dst, logits)
        amax = np.where(np.isfinite(amax), amax, 0.0)
        p = np.exp(logits - amax[dst])
        den = np.zeros((n, heads), np.float32)
        np.add.at(den, dst, p)
        alpha = p / (den[dst] + 1e-16)
        out = np.zeros((n, heads, ch), np.float32)
        np.add.at(out, dst, xl[src] * alpha[..., None])
        out = out.reshape(n, heads * ch) if concat else out.mean(1)
        return out + bias

    h = layer(x, g("Wl1"), g("bl1"), g("Wr1"), g("br1"), g("We1"), g("att1"),
              g("bias1"), H, C, True)
    h = np.where(h > 0, h, np.exp(np.minimum(h, 0)) - 1)
    emb = layer(h, g("Wl2"), g("bl2"), g("Wr2"), g("br2"), g("We2"), g("att2"),
                g("bias2"), 1, C, False)
    z = emb[idx]
    a = np.maximum(z @ g("Wp1") + g("bp1"), 0) @ g("Wp2") + g("bp2")
    v = np.maximum(z @ g("Wv1") + g("bv1"), 0) @ g("Wv2") + g("bv2")
    return a.astype(np.float32), v.astype(np.float32)


def kernel(**inputs):
    dev, dims, lays = _prepare(inputs)
    E1p, ec, n1, E2, degenerate = dims
    if n1 > 128 or E2 > 128:
        return _numpy_fallback(inputs)

    import concourse.bacc as bacc
    from concourse.bass_utils import run_bass_kernel_spmd

    nc = bacc.Bacc("TRN2", target_bir_lowering=False, debug=False)
    _build(nc, dims, lays)
    nc.compile()
    res = run_bass_kernel_spmd(nc, [dict(dev) for _ in range(8)], list(range(8)))
    out = np.asarray(res.results[0]["out"], np.float32).reshape(3)
    return out[:2].copy(), out[2:3].copy()
